# revision 8
# baseline (speedup 1.0000x reference)
"""HSA (hierarchical splat attention) Bass kernel for Trainium2, 8 NeuronCores.

Math (per batch b):
    q = query @ Wq.T + bq                      [S, D]
    v = value @ Wv.T + bv                      [S, D]
    d2[s,n]  = |q_s|^2 - 2 q_s.c_n + |c_n|^2
    G[s,n]   = exp(-d2[s,n] * inv2v[n]),  inv2v = 0.5*exp(-2*log_scales)
    Asym[s,t]= sum_n G[s,n]*amp[n]*G[t,n]      (symmetric!)
    A        = Asym / (rowsum(Asym) + eps)
    out      = A @ v ;  y = out @ Wo.T + bo

Sharding: 8 cores = (batch b = c//2, seq-half h = c%2). Each core computes the
full-batch q-projection/G/v (needed for its rows of A) and its own 1024 output
rows. No collectives. Host pre-transposes inputs so every matmul has its
natural lhsT/rhs layout; the sequence axis is rolled per-core so "own" rows are
always columns 0..1023 (valid since A@v and rowsum are permutation-invariant
over t, and the q-side order is rolled consistently).

Device dataflow (all matmuls are lhsT.T @ rhs, K on partitions):
  qT[e,s]   : lhsT=Wq.T chunk,  rhs=xqT chunk           (accum over d)
  d2T[n,s]  : lhsT=(-2C).T,     rhs=qT   (+ ones64 lhsT, rhs=qT^2 -> |q|^2)
  GT,GampT  : ACT exp with per-partition scale=-inv2v, bias=-inv2v*c2 (+ln amp)
  v[t,e]    : lhsT=xvT chunk,   rhs=Wv.T chunk          (accum over d)
  AsymT[t,s]: lhsT=GT t-chunk,  rhs=GampT own-s   (K=64, one shot)
  rs[s]     : lhsT=ones128,     rhs=AsymT               (accum over t)
  outT[d,s] : lhsT=v d-slice,   rhs=AsymT               (accum over t)
  normalize : outT *= 1/(rs+eps)   (free-dim broadcast tiles)
  y[s,e]    : lhsT=outT s-slice, rhs=Wo.T chunk + bo    (accum over d)
"""

import numpy as np

EMBED = 1024
S = 2048
NSPL = 64
B = 4
NCORES = 8
P = 128
KC = EMBED // P   # 8 contraction chunks over d/e
TCH = S // P      # 16 t-chunks
SOWN = S // 2     # 1024 own output rows per core
SCH = SOWN // P   # 8
EPS = 1e-8

_PROG = None  # cached (nc, input_names)


def _build_program():
    import concourse.bass as bass
    import concourse.mybir as mybir
    from concourse import bacc
    from concourse.tile import TileContext
    from concourse.bass import ts, ds

    f32 = mybir.dt.float32
    AF = mybir.ActivationFunctionType

    nc = bacc.Bacc("TRN2", target_bir_lowering=False, debug=False)
    xqT = nc.declare_dram_parameter("xqT", [EMBED, S], f32, isOutput=False)
    xvT = nc.declare_dram_parameter("xvT", [EMBED, S], f32, isOutput=False)
    wqT = nc.declare_dram_parameter("wqT", [EMBED, EMBED], f32, isOutput=False)
    wvT = nc.declare_dram_parameter("wvT", [EMBED, EMBED], f32, isOutput=False)
    woT = nc.declare_dram_parameter("woT", [EMBED, EMBED], f32, isOutput=False)
    ctm2 = nc.declare_dram_parameter("ctm2", [EMBED, NSPL], f32, isOutput=False)
    bq2 = nc.declare_dram_parameter("bq2", [P, KC], f32, isOutput=False)
    bvb = nc.declare_dram_parameter("bvb", [P, EMBED], f32, isOutput=False)
    bob = nc.declare_dram_parameter("bob", [P, EMBED], f32, isOutput=False)
    scn = nc.declare_dram_parameter("scn", [NSPL, 1], f32, isOutput=False)
    bgn = nc.declare_dram_parameter("bgn", [NSPL, 1], f32, isOutput=False)
    bgan = nc.declare_dram_parameter("bgan", [NSPL, 1], f32, isOutput=False)
    one64 = nc.declare_dram_parameter("one64", [P, NSPL], f32, isOutput=False)
    one128 = nc.declare_dram_parameter("one128", [P, P], f32, isOutput=False)
    y = nc.declare_dram_parameter("y", [SOWN, EMBED], f32, isOutput=True)

    with TileContext(nc) as tc:
        cpool_cm = tc.tile_pool(name="const", bufs=1)
        cpool = cpool_cm.__enter__()
        bq_sb = cpool.tile([P, KC], f32)
        bv_sb = cpool.tile([P, EMBED], f32)
        bo_sb = cpool.tile([P, EMBED], f32)
        sc_sb = cpool.tile([NSPL, 1], f32)
        bg_sb = cpool.tile([NSPL, 1], f32)
        bga_sb = cpool.tile([NSPL, 1], f32)
        o64_sb = cpool.tile([P, NSPL], f32)
        o128_sb = cpool.tile([P, P], f32)
        ct_sb = cpool.tile([P, KC, NSPL], f32)
        gt = cpool.tile([NSPL, S], f32)
        gamp = cpool.tile([NSPL, SOWN], f32)

        nc.sync.dma_start(bq_sb[:], bq2[:])
        nc.sync.dma_start(bv_sb[:], bvb[:])
        nc.sync.dma_start(bo_sb[:], bob[:])
        nc.sync.dma_start(sc_sb[:], scn[:])
        nc.sync.dma_start(bg_sb[:], bgn[:])
        nc.sync.dma_start(bga_sb[:], bgan[:])
        nc.sync.dma_start(o64_sb[:], one64[:])
        nc.sync.dma_start(o128_sb[:], one128[:])
        ctr = ctm2.rearrange("(k p) n -> k p n", p=P)
        for k in range(KC):
            nc.sync.dma_start(ct_sb[:, k], ctr[k])

        # ---------------- Phase A: q projection + G ----------------
        with tc.tile_pool(name="pa", bufs=1) as pa, \
             tc.tile_pool(name="qe", bufs=2) as qep, \
             tc.tile_pool(name="sqe", bufs=2) as sqp, \
             tc.tile_pool(name="psq", bufs=4, space="PSUM") as psq, \
             tc.tile_pool(name="psd2", bufs=4, space="PSUM") as psd2:
            xq = pa.tile([P, KC, S], f32)
            wq = pa.tile([P, KC, EMBED], f32)
            wqr = wqT.rearrange("(k p) e -> k p e", p=P)
            xqr = xqT.rearrange("(k p) s -> k p s", p=P)
            for k in range(KC):
                nc.sync.dma_start(wq[:, k], wqr[k])
                nc.sync.dma_start(xq[:, k], xqr[k])
            d2ps = [psd2.tile([NSPL, 512], f32, tag="d2", name=f"d2ps{i}") for i in range(4)]
            for e in range(KC):
                qps = [psq.tile([P, 512], f32, tag="qps", name=f"qps{e}_{i}") for i in range(4)]
                for k in range(KC):
                    for s4 in range(4):
                        nc.tensor.matmul(
                            qps[s4], wq[:, k, ts(e, P)], xq[:, k, ts(s4, 512)],
                            start=(k == 0), stop=(k == KC - 1))
                qe = qep.tile([P, S], f32, tag="qe")
                for s4 in range(4):
                    nc.scalar.activation(qe[:, ts(s4, 512)], qps[s4],
                                         AF.Identity, bias=bq_sb[:, ds(e, 1)])
                sq = sqp.tile([P, S], f32, tag="sq")
                nc.vector.tensor_mul(sq, qe, qe)
                for s4 in range(4):
                    nc.tensor.matmul(d2ps[s4], ct_sb[:, e], qe[:, ts(s4, 512)],
                                     start=(e == 0), stop=False)
                for s4 in range(4):
                    nc.tensor.matmul(d2ps[s4], o64_sb[:], sq[:, ts(s4, 512)],
                                     start=False, stop=(e == KC - 1))
            for s4 in range(4):
                nc.scalar.activation(gt[:, ts(s4, 512)], d2ps[s4], AF.Exp,
                                     bias=bg_sb[:], scale=sc_sb[:])
            for s2 in range(2):
                nc.scalar.activation(gamp[:, ts(s2, 512)], d2ps[s2], AF.Exp,
                                     bias=bga_sb[:], scale=sc_sb[:])

        # ---------------- Phase B: v projection ----------------
        vpool_cm = tc.tile_pool(name="vpool", bufs=1)
        vpool = vpool_cm.__enter__()
        v_sb = vpool.tile([P, TCH, EMBED], f32)
        with tc.tile_pool(name="pb", bufs=1) as pb, \
             tc.tile_pool(name="psv", bufs=2, space="PSUM") as psv:
            xv = pb.tile([P, KC, S], f32)
            wv = pb.tile([P, KC, EMBED], f32)
            wvr = wvT.rearrange("(k p) e -> k p e", p=P)
            xvr = xvT.rearrange("(k p) s -> k p s", p=P)
            for k in range(KC):
                nc.sync.dma_start(wv[:, k], wvr[k])
                nc.sync.dma_start(xv[:, k], xvr[k])
            for t in range(TCH):
                vps = psv.tile([P, EMBED], f32, tag="vps")
                for k in range(KC):
                    for eh in range(2):
                        nc.tensor.matmul(
                            vps[:, ts(eh, 512)], xv[:, k, ts(t, P)],
                            wv[:, k, ts(eh, 512)],
                            start=(k == 0), stop=(k == KC - 1))
                nc.vector.tensor_add(v_sb[:, t], vps, bv_sb)

        # ---------------- Phase C+D fused: Asym, rowsum, outT ----------------
        wpool_cm = tc.tile_pool(name="wpool", bufs=1)
        wpool = wpool_cm.__enter__()
        wo = wpool.tile([P, KC, EMBED], f32)
        wor = woT.rearrange("(k p) e -> k p e", p=P)
        for k in range(KC):
            nc.sync.dma_start(wo[:, k], wor[k])
        otpool_cm = tc.tile_pool(name="otpool", bufs=1)
        otpool = otpool_cm.__enter__()
        outT = otpool.tile([P, KC, SOWN], f32)

        with tc.tile_pool(name="asym", bufs=3) as asp, \
             tc.tile_pool(name="rssb", bufs=2) as rsp, \
             tc.tile_pool(name="psas", bufs=2, space="PSUM") as psas, \
             tc.tile_pool(name="pso", bufs=4, space="PSUM") as pso, \
             tc.tile_pool(name="psrs", bufs=1, space="PSUM") as psrs:
            for st in range(2):          # own-s tiles of 512
                rsps = psrs.tile([P, 512], f32, tag="rs")
                rsin = None
                for dh in range(2):      # d-chunk halves (4 each)
                    ops = [pso.tile([P, 512], f32, tag="ops", name=f"ops{st}_{dh}_{i}") for i in range(4)]
                    for t in range(TCH):
                        aps = psas.tile([P, 512], f32, tag="aps")
                        nc.tensor.matmul(aps, gt[:, ts(t, P)],
                                         gamp[:, ts(st, 512)],
                                         start=True, stop=True)
                        asy = asp.tile([P, 512], f32, tag="asy")
                        if t % 2 == 0:
                            nc.vector.tensor_copy(asy, aps)
                        else:
                            nc.scalar.activation(asy, aps, AF.Copy)
                        if dh == 0:
                            nc.tensor.matmul(rsps, o128_sb[:], asy,
                                             start=(t == 0), stop=(t == TCH - 1))
                        for i in range(4):
                            d = dh * 4 + i
                            nc.tensor.matmul(ops[i], v_sb[:, t, ts(d, P)], asy,
                                             start=(t == 0), stop=(t == TCH - 1))
                    if dh == 0:
                        rs_sb = rsp.tile([P, 512], f32, tag="rss")
                        nc.vector.tensor_scalar_add(rs_sb, rsps, EPS)
                        rsin = rsp.tile([P, 512], f32, tag="rsin")
                        nc.vector.reciprocal(rsin, rs_sb)
                    for i in range(4):
                        d = dh * 4 + i
                        nc.vector.tensor_mul(outT[:, d, ds(st * 512, 512)],
                                             ops[i], rsin)

        # ---------------- Phase E: output projection ----------------
        with tc.tile_pool(name="ybuf", bufs=2) as yb, \
             tc.tile_pool(name="psy", bufs=2, space="PSUM") as psy:
            yr = y.rearrange("(c p) e -> c p e", p=P)
            for sc in range(SCH):
                yps = psy.tile([P, EMBED], f32, tag="yps")
                for k in range(KC):
                    for eh in range(2):
                        nc.tensor.matmul(
                            yps[:, ts(eh, 512)], outT[:, k, ts(sc, P)],
                            wo[:, k, ts(eh, 512)],
                            start=(k == 0), stop=(k == KC - 1))
                ysb = yb.tile([P, EMBED], f32, tag="ysb")
                nc.vector.tensor_add(ysb, yps, bo_sb)
                nc.sync.dma_start(yr[sc], ysb)
        otpool_cm.__exit__(None, None, None)
        wpool_cm.__exit__(None, None, None)
        vpool_cm.__exit__(None, None, None)
        cpool_cm.__exit__(None, None, None)

    nc.finalize()
    return nc


def _prep_inputs(query, key, value, Wq, bq, Wk, bk, Wv, bv, Wo, bo,
                 splat_centers, splat_log_scales, splat_amplitudes):
    """Build the 8 per-core input maps (host-side sharding/layout prep)."""
    f = np.float32
    q = np.asarray(query, f)
    v = np.asarray(value, f)
    Wq = np.asarray(Wq, f); bq = np.asarray(bq, f)
    Wv = np.asarray(Wv, f); bv = np.asarray(bv, f)
    Wo = np.asarray(Wo, f); bo = np.asarray(bo, f)
    C = np.asarray(splat_centers, f)
    ls = np.asarray(splat_log_scales, f)
    amp = np.asarray(splat_amplitudes, f)

    wqT = np.ascontiguousarray(Wq.T)
    wvT = np.ascontiguousarray(Wv.T)
    woT = np.ascontiguousarray(Wo.T)
    ctm2 = np.ascontiguousarray((-2.0 * C).T)
    bq2 = np.ascontiguousarray(bq.reshape(KC, P).T)
    bvb = np.ascontiguousarray(np.broadcast_to(bv, (P, EMBED)))
    bob = np.ascontiguousarray(np.broadcast_to(bo, (P, EMBED)))
    inv2v = 0.5 * np.exp(-2.0 * ls).astype(f)
    c2 = (C.astype(np.float64) ** 2).sum(1)
    scn = (-inv2v).reshape(NSPL, 1).astype(f)
    bgn = (-inv2v * c2).reshape(NSPL, 1).astype(f)
    # fold amplitude into one G factor: amp*exp(x) = exp(x + ln amp)
    bgan = (-inv2v * c2 + np.log(np.maximum(amp, 1e-38))).reshape(NSPL, 1).astype(f)
    one64 = np.ones((P, NSPL), f)
    one128 = np.ones((P, P), f)

    shared = dict(wqT=wqT, wvT=wvT, woT=woT, ctm2=ctm2, bq2=bq2, bvb=bvb,
                  bob=bob, scn=scn, bgn=bgn, bgan=bgan, one64=one64,
                  one128=one128)
    in_maps = []
    for c in range(NCORES):
        b, h = c // 2, c % 2
        # roll the sequence axis so own rows are always 0..1023
        qb = np.concatenate([q[b, h * SOWN:], q[b, :h * SOWN]], axis=0)
        vb = np.concatenate([v[b, h * SOWN:], v[b, :h * SOWN]], axis=0)
        m = dict(shared)
        m["xqT"] = np.ascontiguousarray(qb.T)
        m["xvT"] = np.ascontiguousarray(vb.T)
        in_maps.append(m)
    return in_maps


def run_cores(inputs, trace=False):
    """Run the SPMD kernel; returns (full_output, BassKernelResults)."""
    global _PROG
    from concourse.bass_utils import run_bass_kernel_spmd
    if _PROG is None:
        _PROG = _build_program()
    nc = _PROG
    in_maps = _prep_inputs(**inputs)
    res = run_bass_kernel_spmd(nc, in_maps, list(range(NCORES)), trace=trace)
    out = np.empty((B, S, EMBED), np.float32)
    for c in range(NCORES):
        b, h = c // 2, c % 2
        out[b, h * SOWN:(h + 1) * SOWN] = res.results[c]["y"]
    return out, res


def kernel(**inputs):
    out, _ = run_cores(inputs, trace=False)
    return out
